# revision 8
# baseline (speedup 1.0000x reference)
"""Distributed LSTM-cell kernel for one TRN2 chip (8 NeuronCores).

Problem: gates = w_ih @ x + b_ih + w_hh @ h_prev + b_hh   (4H x B)
         i,f,g,o = split(gates); c' = sig(f)*c + sig(i)*tanh(g)
         h' = sig(o)*tanh(c'); return sum(c' + h')

Sharding: tensor-parallel over the 4H gate dimension, interleaved so each
core owns rows [d*512,(d+1)*512) of EVERY gate (=> it owns h-rows
[d*512,(d+1)*512) of c'/h').  x / h_prev are replicated.  Each core emits
per-partition partial sums [128, 8]; the host reduces the 8*1024 partials.
No on-chip collective is needed.

Per-core compute: gates_d [2048, 1024] = W_d [2048, 8192] @ [x; h] [8192, 1024].

The final output is a near-cancelling sum (~31 out of 8.4M O(1) terms), so
matmul precision matters enormously: plain bf16 => ~15% rel err.  Schemes:
  bf16x3  split a=hi+lo (bf16); hi@hi + lo@hi + hi@lo      ~1e-4 rel err
  fp32 / fp32r: native fp32 matmul CRASHES the exec unit in this runtime
  and float32r returns garbage -- both unusable here (HW-verified).
  bf16x1/fp16x1  single-pass (accuracy reference only)
"""

import os

import numpy as np

D = 4096
H = 4096
B = 1024
NCORES = 8
RPC = 4 * H // NCORES // 4      # 512 rows per gate per core
HMT = RPC // 128                # 4 h-row tiles of 128 per core
KT = (D + H) // 128             # 64 contraction tiles
NN = B // 512                   # 2 batch halves
P = 128
NBUF = 8                        # stream double-buffer depth (raw kernel)

SCHEME = os.environ.get("LSTM_SCHEME", "bf16x3")

_compiled = {}
LAST_RESULT = None


def _scheme_cfg(scheme):
    import concourse.mybir as mybir

    if scheme == "fp32":
        return dict(dt=mybir.dt.float32, nw=1, nx=1, pairs=[(0, 0)])
    if scheme == "fp32r":
        return dict(dt=mybir.dt.float32r, nw=1, nx=1, pairs=[(0, 0)])
    if scheme == "bf16x1":
        return dict(dt=mybir.dt.bfloat16, nw=1, nx=1, pairs=[(0, 0)])
    if scheme == "fp16x1":
        return dict(dt=mybir.dt.float16, nw=1, nx=1, pairs=[(0, 0)])
    if scheme == "bf16x3":
        return dict(
            dt=mybir.dt.bfloat16, nw=2, nx=2, pairs=[(0, 0), (1, 0), (0, 1)]
        )
    raise ValueError(scheme)


def _build(scheme):
    import concourse.bacc as bacc
    import concourse.mybir as mybir
    from concourse.tile import TileContext

    cfg = _scheme_cfg(scheme)
    dt_mm = cfg["dt"]
    pairs = cfg["pairs"]
    nsrc = len(pairs)
    f32 = mybir.dt.float32
    AFT = mybir.ActivationFunctionType
    ALU = mybir.AluOpType

    nc = bacc.Bacc("TRN2", target_bir_lowering=False, debug=False)

    w_drams = [
        nc.dram_tensor(f"wt{i}", [HMT, KT * P, 512], dt_mm, kind="ExternalInput")
        for i in range(cfg["nw"])
    ]
    x_drams = [
        nc.dram_tensor(f"xh{i}", [KT * P, B], dt_mm, kind="ExternalInput")
        for i in range(cfg["nx"])
    ]
    cprev_d = nc.dram_tensor("cprev", [RPC, B], f32, kind="ExternalInput")
    bias_d = nc.dram_tensor("bias", [P, HMT * 4], f32, kind="ExternalInput")
    out_d = nc.dram_tensor("out", [P, HMT * NN], f32, kind="ExternalOutput")

    with TileContext(nc) as tc:
        with (
            tc.tile_pool(name="wp", bufs=4) as wp,
            tc.tile_pool(name="xp", bufs=4) as xp,
            tc.tile_pool(name="pp", bufs=8, space="PSUM") as pp,
            tc.tile_pool(name="ep", bufs=2) as ep,
            tc.tile_pool(name="mp", bufs=1) as mp,
        ):
            bias_sb = mp.tile([P, HMT * 4], f32, tag="bias")
            nc.sync.dma_start(out=bias_sb[:, :], in_=bias_d[:, :])
            acc_sb = mp.tile([P, HMT * NN], f32, tag="acc")

            for hm in range(HMT):
                ps = [pp.tile([P, 512], f32, tag="ps", name=f"ps{hm}_{j}") for j in range(8)]
                for kt in range(KT):
                    wt = [
                        wp.tile([P, 512], dt_mm, tag=f"w{i}", name=f"w{i}_{hm}_{kt}")
                        for i in range(cfg["nw"])
                    ]
                    for i in range(cfg["nw"]):
                        nc.sync.dma_start(
                            out=wt[i][:, :],
                            in_=w_drams[i][hm, kt * P : (kt + 1) * P, :],
                        )
                    xt = [
                        xp.tile([P, B], dt_mm, tag=f"x{i}", name=f"x{i}_{hm}_{kt}")
                        for i in range(cfg["nx"])
                    ]
                    for i in range(cfg["nx"]):
                        nc.sync.dma_start(
                            out=xt[i][:, :],
                            in_=x_drams[i][kt * P : (kt + 1) * P, :],
                        )
                    for g in range(4):
                        for si, (wi, xi) in enumerate(pairs):
                            lhsT = wt[wi][:, g * P : (g + 1) * P]
                            for n in range(NN):
                                nc.tensor.matmul(
                                    ps[g * NN + n][:, :],
                                    lhsT,
                                    xt[xi][:, n * 512 : (n + 1) * 512],
                                    start=(kt == 0 and si == 0),
                                    stop=(kt == KT - 1 and si == nsrc - 1),
                                )

                for n in range(NN):
                    idx = hm * NN + n
                    cp = ep.tile([P, 512], f32, tag="cp")
                    nc.sync.dma_start(
                        out=cp[:, :],
                        in_=cprev_d[hm * P : (hm + 1) * P, n * 512 : (n + 1) * 512],
                    )
                    i_sb = ep.tile([P, 512], f32, tag="i")
                    f_sb = ep.tile([P, 512], f32, tag="f")
                    g_sb = ep.tile([P, 512], f32, tag="g")
                    o_sb = ep.tile([P, 512], f32, tag="o")
                    for t_sb, gi, fn in (
                        (i_sb, 0, AFT.Sigmoid),
                        (f_sb, 1, AFT.Sigmoid),
                        (g_sb, 2, AFT.Tanh),
                        (o_sb, 3, AFT.Sigmoid),
                    ):
                        nc.scalar.activation(
                            t_sb[:, :],
                            ps[gi * NN + n][:, :],
                            fn,
                            bias=bias_sb[:, hm * 4 + gi : hm * 4 + gi + 1],
                        )
                    t_fc = ep.tile([P, 512], f32, tag="fc")
                    nc.vector.tensor_mul(t_fc[:, :], f_sb[:, :], cp[:, :])
                    t_ig = ep.tile([P, 512], f32, tag="ig")
                    nc.vector.tensor_mul(t_ig[:, :], i_sb[:, :], g_sb[:, :])
                    t_c = ep.tile([P, 512], f32, tag="c")
                    nc.vector.tensor_add(t_c[:, :], t_fc[:, :], t_ig[:, :])
                    t_tc = ep.tile([P, 512], f32, tag="tc")
                    nc.scalar.activation(t_tc[:, :], t_c[:, :], AFT.Tanh)
                    t_h = ep.tile([P, 512], f32, tag="h")
                    nc.vector.tensor_mul(t_h[:, :], o_sb[:, :], t_tc[:, :])
                    t_s = ep.tile([P, 512], f32, tag="s")
                    nc.vector.tensor_add(t_s[:, :], t_c[:, :], t_h[:, :])
                    nc.vector.reduce_sum(
                        acc_sb[:, idx : idx + 1],
                        t_s[:, :],
                        axis=mybir.AxisListType.X,
                    )

            nc.sync.dma_start(out=out_d[:, :], in_=acc_sb[:, :])

    nc.compile()
    return nc


def _build_raw():
    import concourse.bacc as bacc
    import concourse.mybir as mybir

    dt = mybir.dt.bfloat16
    f32 = mybir.dt.float32
    AFT = mybir.ActivationFunctionType
    ALU = mybir.AluOpType

    nc = bacc.Bacc("TRN2", target_bir_lowering=False, debug=False)

    w_drams = [
        nc.dram_tensor(f"wt{i}", [HMT, KT * P, 512], dt, kind="ExternalInput")
        for i in range(2)
    ]
    x_drams = [
        nc.dram_tensor(f"xh{i}", [KT * P, B], dt, kind="ExternalInput")
        for i in range(2)
    ]
    cprev_d = nc.dram_tensor("cprev", [RPC, B], f32, kind="ExternalInput")
    bias_d = nc.dram_tensor("bias", [P, HMT * 4], f32, kind="ExternalInput")
    out_d = nc.dram_tensor("out", [P, HMT * NN], f32, kind="ExternalOutput")

    NG = HMT * KT  # 256 k-tile groups

    from contextlib import ExitStack

    with ExitStack() as ctx:
        e = ctx.enter_context
        whi = e(nc.sbuf_tensor([P, NBUF, 512], dt))
        wlo = e(nc.sbuf_tensor([P, NBUF, 512], dt))
        xhi = e(nc.sbuf_tensor([P, NBUF, 1024], dt))
        xlo = e(nc.sbuf_tensor([P, NBUF, 1024], dt))
        cp = e(nc.sbuf_tensor([P, HMT * NN, 512], f32))
        bias_sb = e(nc.sbuf_tensor([P, HMT * 4], f32))
        acc_sb = e(nc.sbuf_tensor([P, HMT * NN], f32))
        ps = e(nc.psum_tensor([P, 8, 512], f32))
        i_sb = e(nc.sbuf_tensor([P, NN, 512], f32))
        f_sb = e(nc.sbuf_tensor([P, NN, 512], f32))
        g_sb = e(nc.sbuf_tensor([P, NN, 512], f32))
        o_sb = e(nc.sbuf_tensor([P, NN, 512], f32))
        t_fc = e(nc.sbuf_tensor([P, NN, 512], f32))
        t_ig = e(nc.sbuf_tensor([P, NN, 512], f32))
        t_c = e(nc.sbuf_tensor([P, NN, 512], f32))
        t_tc = e(nc.sbuf_tensor([P, NN, 512], f32))
        t_h = e(nc.sbuf_tensor([P, NN, 512], f32))
        t_s = e(nc.sbuf_tensor([P, NN, 512], f32))
        dsems = [e(nc.semaphore(f"dsem{j}")) for j in range(NBUF)]
        init_sem = e(nc.semaphore("init_sem"))
        pe_kt = e(nc.semaphore("pe_kt"))
        a_sem = e(nc.semaphore("a_sem"))
        d_sem = e(nc.semaphore("d_sem"))
        block = e(nc.Block())

        DMA_INIT = 1 + HMT * NN  # bias + cprev tiles

        @block.sync
        def _(sync):
            def init_dmas():
                sync.dma_start(out=bias_sb[:, :], in_=bias_d[:, :]).then_inc(
                    init_sem, 16
                )
                for hm in range(HMT):
                    for n in range(NN):
                        sync.dma_start(
                            out=cp[:, hm * NN + n, :],
                            in_=cprev_d[
                                hm * P : (hm + 1) * P, n * 512 : (n + 1) * 512
                            ],
                        ).then_inc(init_sem, 16)

            for gi in range(NG):
                if gi == NBUF:
                    # init tensors aren't needed until the first epilogue;
                    # issue them after the stream pipeline is primed so kt0
                    # isn't stuck behind 9 serial ring transfers.
                    init_dmas()
                hm, kt = divmod(gi, KT)
                slot = gi % NBUF
                if gi >= NBUF:
                    sync.wait_ge(pe_kt, gi - NBUF + 1)
                sync.dma_start(
                    out=whi[:, slot, :],
                    in_=w_drams[0][hm, kt * P : (kt + 1) * P, :],
                ).then_inc(dsems[slot], 16)
                sync.dma_start(
                    out=wlo[:, slot, :],
                    in_=w_drams[1][hm, kt * P : (kt + 1) * P, :],
                ).then_inc(dsems[slot], 16)
                sync.dma_start(
                    out=xhi[:, slot, :], in_=x_drams[0][kt * P : (kt + 1) * P, :]
                ).then_inc(dsems[slot], 16)
                sync.dma_start(
                    out=xlo[:, slot, :], in_=x_drams[1][kt * P : (kt + 1) * P, :]
                ).then_inc(dsems[slot], 16)
            sync.wait_ge(d_sem, 12 * HMT)
            sync.dma_start(out=out_d[:, :], in_=acc_sb[:, :]).then_inc(init_sem, 16)

        @block.tensor
        def _(tensor):
            for hm in range(HMT):
                for kt in range(KT):
                    gi = hm * KT + kt
                    slot = gi % NBUF
                    # slot-sem threshold: use-(gi//NBUF) of this slot fully
                    # DMA'd.  Unambiguous even with unordered DMA completion:
                    # the next use of this slot is issued only after SP's WAR
                    # wait on pe_kt, which itself requires this wait to pass.
                    tensor.wait_ge(dsems[slot], 64 * (gi // NBUF + 1))
                    mm = None
                    for g in range(4):
                        for si, (wt, xt) in enumerate(
                            ((whi, xhi), (wlo, xhi), (whi, xlo))
                        ):
                            lhsT = wt[:, slot, g * P : (g + 1) * P]
                            for n in range(NN):
                                if kt == 0 and si == 0 and hm > 0:
                                    # per-bank WAR wait: ACT of prev phase must
                                    # have read this bank (act inc 1+g / 5+g)
                                    v = 10 * (hm - 1) + (1 + g if n == 0 else 5 + g)
                                    tensor.wait_ge(a_sem, v)
                                mm = nc.tensor.matmul(
                                    ps[:, g * NN + n, :],
                                    lhsT,
                                    xt[:, slot, n * 512 : (n + 1) * 512],
                                    start=(kt == 0 and si == 0),
                                    stop=(kt == KT - 1 and si == 2),
                                )
                    mm.then_inc(pe_kt, 1)

        @block.scalar
        def _(scalar):
            scalar.wait_ge(init_sem, 16 * DMA_INIT)  # bias loaded
            for hm in range(HMT):
                scalar.wait_ge(pe_kt, KT * (hm + 1))
                # 8 gate activations (frees PSUM banks in order), then 2 tanh(c)
                for n in range(NN):
                    if hm > 0:
                        scalar.wait_ge(d_sem, 12 * (hm - 1) + (7 if n == 0 else 10))
                    for g, (t_sb, fn) in enumerate(
                        (
                            (i_sb, AFT.Sigmoid),
                            (f_sb, AFT.Sigmoid),
                            (g_sb, AFT.Tanh),
                            (o_sb, AFT.Sigmoid),
                        )
                    ):
                        nc.scalar.activation(
                            t_sb[:, n, :],
                            ps[:, g * NN + n, :],
                            fn,
                            bias=bias_sb[:, hm * 4 + g : hm * 4 + g + 1],
                        ).then_inc(a_sem, 1)
                for n in range(NN):
                    scalar.wait_ge(d_sem, 12 * hm + (3 if n == 0 else 6))
                    nc.scalar.activation(
                        t_tc[:, n, :], t_c[:, n, :], AFT.Tanh
                    ).then_inc(a_sem, 1)

        @block.vector
        def _(vector):
            vector.wait_ge(init_sem, 16 * DMA_INIT)  # cprev tiles loaded
            for hm in range(HMT):
                base = 10 * hm
                for n in range(NN):
                    vector.wait_ge(a_sem, base + (2 if n == 0 else 6))
                    nc.vector.tensor_mul(
                        t_fc[:, n, :], f_sb[:, n, :], cp[:, hm * NN + n, :]
                    ).then_inc(d_sem, 1)
                    vector.wait_ge(a_sem, base + (3 if n == 0 else 7))
                    nc.vector.tensor_mul(
                        t_ig[:, n, :], i_sb[:, n, :], g_sb[:, n, :]
                    ).then_inc(d_sem, 1)
                    nc.vector.tensor_add(
                        t_c[:, n, :], t_fc[:, n, :], t_ig[:, n, :]
                    ).then_inc(d_sem, 1)
                for n in range(NN):
                    vector.wait_ge(a_sem, base + (9 if n == 0 else 10))
                    nc.vector.tensor_mul(
                        t_h[:, n, :], o_sb[:, n, :], t_tc[:, n, :]
                    ).then_inc(d_sem, 1)
                    nc.vector.tensor_add(
                        t_s[:, n, :], t_c[:, n, :], t_h[:, n, :]
                    ).then_inc(d_sem, 1)
                    idx = hm * NN + n
                    nc.vector.reduce_sum(
                        acc_sb[:, idx : idx + 1],
                        t_s[:, n, :],
                        axis=mybir.AxisListType.X,
                    ).then_inc(d_sem, 1)

    nc.compile()
    return nc



def _get_compiled(scheme):
    if scheme not in _compiled:
        if scheme == "bf16x3":
            _compiled[scheme] = _build_raw()
        elif scheme == "bf16x3_tile":
            _compiled[scheme] = _build("bf16x3")
        else:
            _compiled[scheme] = _build(scheme)
    return _compiled[scheme]


def _split_lohi(a, np_dt):
    hi = a.astype(np_dt)
    lo = (a - hi.astype(np.float32)).astype(np_dt)
    return hi, lo


def _prep_inputs(scheme, x, h_prev, c_prev, w_ih, w_hh, b_ih, b_hh):
    import ml_dtypes

    f32 = np.float32
    x = np.asarray(x, f32)
    h_prev = np.asarray(h_prev, f32)
    c_prev = np.asarray(c_prev, f32)
    w_ih = np.asarray(w_ih, f32)
    w_hh = np.asarray(w_hh, f32)
    b = (np.asarray(b_ih, f32) + np.asarray(b_hh, f32)).reshape(4, NCORES, HMT, P)

    xh = np.concatenate([x, h_prev], axis=0)  # [8192, B]

    if scheme == "bf16x3_tile":
        scheme = "bf16x3"
    if scheme in ("fp32", "fp32r"):
        np_dt = f32
    elif scheme in ("bf16x1", "bf16x3"):
        np_dt = ml_dtypes.bfloat16
    elif scheme == "fp16x1":
        np_dt = np.float16
    else:
        raise ValueError(scheme)

    split = scheme.endswith("x3")
    if split:
        xh_hi, xh_lo = _split_lohi(xh, np_dt)
        x_maps = {"xh0": xh_hi, "xh1": xh_lo}
    else:
        x_maps = {"xh0": xh.astype(np_dt)}

    wih_r = w_ih.reshape(4, NCORES, RPC, D)
    whh_r = w_hh.reshape(4, NCORES, RPC, H)

    in_maps = []
    for d in range(NCORES):
        wc = np.concatenate([wih_r[:, d], whh_r[:, d]], axis=2)  # (4, 512, 8192)
        wc = wc.reshape(4, HMT, P, D + H)  # (g, hm, r, k)
        wt = np.ascontiguousarray(wc.transpose(1, 3, 0, 2)).reshape(
            HMT, D + H, 4 * P
        )  # (hm, k, g*128+r)
        m = dict(x_maps)
        if split:
            w_hi, w_lo = _split_lohi(wt, np_dt)
            m["wt0"] = w_hi
            m["wt1"] = w_lo
        else:
            m["wt0"] = wt.astype(np_dt)
        m["cprev"] = np.ascontiguousarray(c_prev[d * RPC : (d + 1) * RPC])
        m["bias"] = np.ascontiguousarray(
            b[:, d].transpose(2, 1, 0).reshape(P, HMT * 4)
        )
        in_maps.append(m)
    return in_maps


def _ensure_axon_ntff_hook():
    """Register the axon NTFF-profile hook if the container's `antenv` stub
    lacks `axon_hooks` (needed only for trace=True / BASS_TRACE runs)."""
    import contextlib
    import ctypes
    import sys
    import types

    try:
        from antenv import axon_hooks  # noqa: F401

        return
    except ImportError:
        pass
    try:
        import antenv
    except ImportError:
        return

    holder = {}
    mod = types.ModuleType("antenv.axon_hooks")
    mod.set_axon_ntff_profile_hook = lambda h: holder.__setitem__("h", h)
    mod.get_axon_ntff_profile_hook = lambda: holder.get("h")
    sys.modules["antenv.axon_hooks"] = mod
    antenv.axon_hooks = mod

    so_path = "/opt/axon/libaxon_pjrt.so"
    try:
        lib = ctypes.CDLL(so_path)
        if not hasattr(lib, "axon_start_nrt_profile"):
            return
        lib.axon_start_nrt_profile.argtypes = [
            ctypes.POINTER(ctypes.c_int64),
            ctypes.c_size_t,
        ]
        lib.axon_start_nrt_profile.restype = ctypes.c_int64
        lib.axon_stop_nrt_profile.argtypes = [ctypes.c_char_p]
        lib.axon_stop_nrt_profile.restype = ctypes.c_int64

        @contextlib.contextmanager
        def _hook(output_dir, device_ids):
            import jax

            jax.devices()
            if device_ids:
                ids = (ctypes.c_int64 * len(device_ids))(*device_ids)
                rc = lib.axon_start_nrt_profile(ids, len(device_ids))
            else:
                rc = lib.axon_start_nrt_profile(None, 0)
            if rc != 0:
                raise RuntimeError(f"axon_start_nrt_profile rc={rc}")
            try:
                yield
            finally:
                n = lib.axon_stop_nrt_profile(str(output_dir).encode())
                print(f"ntff profile: {n} file(s) -> {output_dir}", file=sys.stderr)

        mod.set_axon_ntff_profile_hook(_hook)
    except Exception:
        pass


def kernel(x, h_prev, c_prev, w_ih, w_hh, b_ih, b_hh):
    global LAST_RESULT
    from concourse.bass_utils import run_bass_kernel_spmd

    if os.environ.get("BASS_TRACE"):
        _ensure_axon_ntff_hook()

    scheme = SCHEME
    nc = _get_compiled(scheme)
    in_maps = _prep_inputs(scheme, x, h_prev, c_prev, w_ih, w_hh, b_ih, b_hh)
    res = run_bass_kernel_spmd(nc, in_maps, core_ids=list(range(NCORES)))
    LAST_RESULT = res
    total = np.float64(0.0)
    for r in res.results:
        total += np.asarray(r["out"], np.float64).sum()
    return np.array(total, dtype=np.float32)


# revision 13
# speedup vs baseline: 1.2081x; 1.2081x over previous
"""Distributed LSTM-cell kernel for one TRN2 chip (8 NeuronCores).

Problem: gates = w_ih @ x + b_ih + w_hh @ h_prev + b_hh   (4H x B)
         i,f,g,o = split(gates); c' = sig(f)*c + sig(i)*tanh(g)
         h' = sig(o)*tanh(c'); return sum(c' + h')

Sharding: tensor-parallel over the 4H gate dimension, interleaved so each
core owns rows [d*512,(d+1)*512) of EVERY gate (=> it owns h-rows
[d*512,(d+1)*512) of c'/h').  x / h_prev are replicated.  Each core emits
per-partition partial sums [128, 8]; the host reduces the 8*1024 partials.
No on-chip collective is needed.

Per-core compute: gates_d [2048, 1024] = W_d [2048, 8192] @ [x; h] [8192, 1024].

The final output is a near-cancelling sum (~31 out of 8.4M O(1) terms), so
matmul precision matters enormously: plain bf16 => ~15% rel err.  Schemes:
  bf16x3  split a=hi+lo (bf16); hi@hi + lo@hi + hi@lo      ~1e-4 rel err
  fp32 / fp32r: native fp32 matmul CRASHES the exec unit in this runtime
  and float32r returns garbage -- both unusable here (HW-verified).
  bf16x1/fp16x1  single-pass (accuracy reference only)
"""

import os

import numpy as np

D = 4096
H = 4096
B = 1024
NCORES = 8
RPC = 4 * H // NCORES // 4      # 512 rows per gate per core
HMT = RPC // 128                # 4 h-row tiles of 128 per core
KT = (D + H) // 128             # 64 contraction tiles
NN = B // 512                   # 2 batch halves
P = 128
NBUF = 8                        # stream double-buffer depth (raw kernel)

SCHEME = os.environ.get("LSTM_SCHEME", "bf16x3")

_compiled = {}
LAST_RESULT = None


def _scheme_cfg(scheme):
    import concourse.mybir as mybir

    if scheme == "fp32":
        return dict(dt=mybir.dt.float32, nw=1, nx=1, pairs=[(0, 0)])
    if scheme == "fp32r":
        return dict(dt=mybir.dt.float32r, nw=1, nx=1, pairs=[(0, 0)])
    if scheme == "bf16x1":
        return dict(dt=mybir.dt.bfloat16, nw=1, nx=1, pairs=[(0, 0)])
    if scheme == "fp16x1":
        return dict(dt=mybir.dt.float16, nw=1, nx=1, pairs=[(0, 0)])
    if scheme == "bf16x3":
        return dict(
            dt=mybir.dt.bfloat16, nw=2, nx=2, pairs=[(0, 0), (1, 0), (0, 1)]
        )
    raise ValueError(scheme)


def _build(scheme):
    import concourse.bacc as bacc
    import concourse.mybir as mybir
    from concourse.tile import TileContext

    cfg = _scheme_cfg(scheme)
    dt_mm = cfg["dt"]
    pairs = cfg["pairs"]
    nsrc = len(pairs)
    f32 = mybir.dt.float32
    AFT = mybir.ActivationFunctionType
    ALU = mybir.AluOpType

    nc = bacc.Bacc("TRN2", target_bir_lowering=False, debug=False)

    w_drams = [
        nc.dram_tensor(f"wt{i}", [HMT, KT * P, 512], dt_mm, kind="ExternalInput")
        for i in range(cfg["nw"])
    ]
    x_drams = [
        nc.dram_tensor(f"xh{i}", [KT * P, B], dt_mm, kind="ExternalInput")
        for i in range(cfg["nx"])
    ]
    cprev_d = nc.dram_tensor("cprev", [RPC, B], f32, kind="ExternalInput")
    bias_d = nc.dram_tensor("bias", [P, HMT * 4], f32, kind="ExternalInput")
    out_d = nc.dram_tensor("out", [P, HMT * NN], f32, kind="ExternalOutput")

    with TileContext(nc) as tc:
        with (
            tc.tile_pool(name="wp", bufs=4) as wp,
            tc.tile_pool(name="xp", bufs=4) as xp,
            tc.tile_pool(name="pp", bufs=8, space="PSUM") as pp,
            tc.tile_pool(name="ep", bufs=2) as ep,
            tc.tile_pool(name="mp", bufs=1) as mp,
        ):
            bias_sb = mp.tile([P, HMT * 4], f32, tag="bias")
            nc.sync.dma_start(out=bias_sb[:, :], in_=bias_d[:, :])
            acc_sb = mp.tile([P, HMT * NN], f32, tag="acc")

            for hm in range(HMT):
                ps = [pp.tile([P, 512], f32, tag="ps", name=f"ps{hm}_{j}") for j in range(8)]
                for kt in range(KT):
                    wt = [
                        wp.tile([P, 512], dt_mm, tag=f"w{i}", name=f"w{i}_{hm}_{kt}")
                        for i in range(cfg["nw"])
                    ]
                    for i in range(cfg["nw"]):
                        nc.sync.dma_start(
                            out=wt[i][:, :],
                            in_=w_drams[i][hm, kt * P : (kt + 1) * P, :],
                        )
                    xt = [
                        xp.tile([P, B], dt_mm, tag=f"x{i}", name=f"x{i}_{hm}_{kt}")
                        for i in range(cfg["nx"])
                    ]
                    for i in range(cfg["nx"]):
                        nc.sync.dma_start(
                            out=xt[i][:, :],
                            in_=x_drams[i][kt * P : (kt + 1) * P, :],
                        )
                    for g in range(4):
                        for si, (wi, xi) in enumerate(pairs):
                            lhsT = wt[wi][:, g * P : (g + 1) * P]
                            for n in range(NN):
                                nc.tensor.matmul(
                                    ps[g * NN + n][:, :],
                                    lhsT,
                                    xt[xi][:, n * 512 : (n + 1) * 512],
                                    start=(kt == 0 and si == 0),
                                    stop=(kt == KT - 1 and si == nsrc - 1),
                                )

                for n in range(NN):
                    idx = hm * NN + n
                    cp = ep.tile([P, 512], f32, tag="cp")
                    nc.sync.dma_start(
                        out=cp[:, :],
                        in_=cprev_d[hm * P : (hm + 1) * P, n * 512 : (n + 1) * 512],
                    )
                    i_sb = ep.tile([P, 512], f32, tag="i")
                    f_sb = ep.tile([P, 512], f32, tag="f")
                    g_sb = ep.tile([P, 512], f32, tag="g")
                    o_sb = ep.tile([P, 512], f32, tag="o")
                    for t_sb, gi, fn in (
                        (i_sb, 0, AFT.Sigmoid),
                        (f_sb, 1, AFT.Sigmoid),
                        (g_sb, 2, AFT.Tanh),
                        (o_sb, 3, AFT.Sigmoid),
                    ):
                        nc.scalar.activation(
                            t_sb[:, :],
                            ps[gi * NN + n][:, :],
                            fn,
                            bias=bias_sb[:, hm * 4 + gi : hm * 4 + gi + 1],
                        )
                    t_fc = ep.tile([P, 512], f32, tag="fc")
                    nc.vector.tensor_mul(t_fc[:, :], f_sb[:, :], cp[:, :])
                    t_ig = ep.tile([P, 512], f32, tag="ig")
                    nc.vector.tensor_mul(t_ig[:, :], i_sb[:, :], g_sb[:, :])
                    t_c = ep.tile([P, 512], f32, tag="c")
                    nc.vector.tensor_add(t_c[:, :], t_fc[:, :], t_ig[:, :])
                    t_tc = ep.tile([P, 512], f32, tag="tc")
                    nc.scalar.activation(t_tc[:, :], t_c[:, :], AFT.Tanh)
                    t_h = ep.tile([P, 512], f32, tag="h")
                    nc.vector.tensor_mul(t_h[:, :], o_sb[:, :], t_tc[:, :])
                    t_s = ep.tile([P, 512], f32, tag="s")
                    nc.vector.tensor_add(t_s[:, :], t_c[:, :], t_h[:, :])
                    nc.vector.reduce_sum(
                        acc_sb[:, idx : idx + 1],
                        t_s[:, :],
                        axis=mybir.AxisListType.X,
                    )

            nc.sync.dma_start(out=out_d[:, :], in_=acc_sb[:, :])

    nc.compile()
    return nc


def _build_raw():
    import concourse.bacc as bacc
    import concourse.mybir as mybir

    dt = mybir.dt.bfloat16
    f32 = mybir.dt.float32
    AFT = mybir.ActivationFunctionType
    ALU = mybir.AluOpType

    nc = bacc.Bacc("TRN2", target_bir_lowering=False, debug=False)

    w_drams = [
        nc.dram_tensor(f"wt{i}", [HMT, KT * P, 512], dt, kind="ExternalInput")
        for i in range(2)
    ]
    x_drams = [
        nc.dram_tensor(f"xh{i}", [KT * P, B], dt, kind="ExternalInput")
        for i in range(2)
    ]
    cprev_d = nc.dram_tensor("cprev", [RPC, B], f32, kind="ExternalInput")
    bias_d = nc.dram_tensor("bias", [P, HMT * 4], f32, kind="ExternalInput")
    out_d = nc.dram_tensor("out", [P, HMT * NN], f32, kind="ExternalOutput")

    NG = HMT * KT  # 256 k-tile groups

    from contextlib import ExitStack

    with ExitStack() as ctx:
        e = ctx.enter_context
        whi = e(nc.sbuf_tensor([P, NBUF, 512], dt))
        wlo = e(nc.sbuf_tensor([P, NBUF, 512], dt))
        xhi = e(nc.sbuf_tensor([P, NBUF, 1024], dt))
        xlo = e(nc.sbuf_tensor([P, NBUF, 1024], dt))
        cp = e(nc.sbuf_tensor([P, HMT * NN, 512], f32))
        bias_sb = e(nc.sbuf_tensor([P, HMT * 4], f32))
        acc_sb = e(nc.sbuf_tensor([P, HMT * NN], f32))
        ps = e(nc.psum_tensor([P, 8, 512], f32))
        i_sb = e(nc.sbuf_tensor([P, NN, 512], f32))
        f_sb = e(nc.sbuf_tensor([P, NN, 512], f32))
        g_sb = e(nc.sbuf_tensor([P, NN, 512], f32))
        o_sb = e(nc.sbuf_tensor([P, NN, 512], f32))
        t_fc = e(nc.sbuf_tensor([P, NN, 512], f32))
        t_ig = e(nc.sbuf_tensor([P, NN, 512], f32))
        t_c = e(nc.sbuf_tensor([P, NN, 512], f32))
        t_tc = e(nc.sbuf_tensor([P, NN, 512], f32))
        t_h = e(nc.sbuf_tensor([P, NN, 512], f32))
        t_s = e(nc.sbuf_tensor([P, NN, 512], f32))
        dsems = [e(nc.semaphore(f"dsem{j}")) for j in range(NBUF)]
        init_sem = e(nc.semaphore("init_sem"))
        pe_bank = e(nc.semaphore("pe_bank"))
        pe_kt = e(nc.semaphore("pe_kt"))
        a_sem = e(nc.semaphore("a_sem"))
        d_sem = e(nc.semaphore("d_sem"))
        block = e(nc.Block(no_gpsimd_drain=True))

        DMA_INIT = 1 + HMT * NN  # bias + cprev tiles

        @block.sync
        def _(sync):
            def init_dmas():
                sync.dma_start(out=bias_sb[:, :], in_=bias_d[:, :]).then_inc(
                    init_sem, 16
                )
                for hm in range(HMT):
                    for n in range(NN):
                        sync.dma_start(
                            out=cp[:, hm * NN + n, :],
                            in_=cprev_d[
                                hm * P : (hm + 1) * P, n * 512 : (n + 1) * 512
                            ],
                        ).then_inc(init_sem, 16)

            for gi in range(NG):
                if gi == NBUF:
                    # init tensors aren't needed until the first epilogue;
                    # issue them after the stream pipeline is primed so kt0
                    # isn't stuck behind 9 serial ring transfers.
                    init_dmas()
                hm, kt = divmod(gi, KT)
                slot = gi % NBUF
                if gi >= NBUF:
                    sync.wait_ge(pe_kt, gi - NBUF + 1)
                sync.dma_start(
                    out=whi[:, slot, :],
                    in_=w_drams[0][hm, kt * P : (kt + 1) * P, :],
                ).then_inc(dsems[slot], 16)
                sync.dma_start(
                    out=wlo[:, slot, :],
                    in_=w_drams[1][hm, kt * P : (kt + 1) * P, :],
                ).then_inc(dsems[slot], 16)
                sync.dma_start(
                    out=xhi[:, slot, :], in_=x_drams[0][kt * P : (kt + 1) * P, :]
                ).then_inc(dsems[slot], 16)
                sync.dma_start(
                    out=xlo[:, slot, :], in_=x_drams[1][kt * P : (kt + 1) * P, :]
                ).then_inc(dsems[slot], 16)
            sync.wait_ge(d_sem, 12 * HMT)
            sync.dma_start(out=out_d[:, :], in_=acc_sb[:, :]).then_inc(init_sem, 16)

        @block.tensor
        def _(tensor):
            for hm in range(HMT):
                for kt in range(KT):
                    gi = hm * KT + kt
                    slot = gi % NBUF
                    # slot-sem threshold: use-(gi//NBUF) of this slot fully
                    # DMA'd.  Unambiguous even with unordered DMA completion:
                    # the next use of this slot is issued only after SP's WAR
                    # wait on pe_kt, which itself requires this wait to pass.
                    tensor.wait_ge(dsems[slot], 64 * (gi // NBUF + 1))
                    mm = None
                    for g in range(4):
                        for si, (wt, xt) in enumerate(
                            ((whi, xhi), (wlo, xhi), (whi, xlo))
                        ):
                            lhsT = wt[:, slot, g * P : (g + 1) * P]
                            for n in range(NN):
                                if kt == 0 and si == 0 and hm > 0:
                                    # per-bank WAR wait: ACT of prev phase must
                                    # have read this bank (act inc 1+g / 5+g)
                                    v = 10 * (hm - 1) + (1 + g if n == 0 else 5 + g)
                                    tensor.wait_ge(a_sem, v)
                                mm = nc.tensor.matmul(
                                    ps[:, g * NN + n, :],
                                    lhsT,
                                    xt[:, slot, n * 512 : (n + 1) * 512],
                                    start=(kt == 0 and si == 0),
                                    stop=(kt == KT - 1 and si == 2),
                                )
                                if (
                                    kt == KT - 1
                                    and si == 2
                                    and not (g == 3 and n == NN - 1)
                                ):
                                    # bank (g, n) fully accumulated: let ACT
                                    # start this gate's activation while the
                                    # remaining banks still stream.  The very
                                    # last bank signals via pe_kt instead (a
                                    # MM can carry only one sem update).
                                    mm.then_inc(pe_bank, 1)
                    mm.then_inc(pe_kt, 1)

        @block.scalar
        def _(scalar):
            scalar.wait_ge(init_sem, 16 * DMA_INIT)  # bias loaded
            for hm in range(HMT):
                # 8 gate activations (frees PSUM banks in order), then 2 tanh(c)
                for n in range(NN):
                    if hm > 0:
                        scalar.wait_ge(d_sem, 12 * (hm - 1) + (7 if n == 0 else 10))
                    for g, (t_sb, fn) in enumerate(
                        (
                            (i_sb, AFT.Sigmoid),
                            (f_sb, AFT.Sigmoid),
                            (g_sb, AFT.Tanh),
                            (o_sb, AFT.Sigmoid),
                        )
                    ):
                        # bank (g, n) done once its stop-MM retired; stop-MM
                        # inc order within the last k-tile is (g0 n0), (g0 n1),
                        # (g1 n0), ... => index 2g + n + 1 (7 incs/phase; the
                        # last bank (g3, n1) signals via pe_kt)
                        if g == 3 and n == NN - 1:
                            scalar.wait_ge(pe_kt, KT * (hm + 1))
                        else:
                            scalar.wait_ge(pe_bank, 7 * hm + 2 * g + n + 1)
                        nc.scalar.activation(
                            t_sb[:, n, :],
                            ps[:, g * NN + n, :],
                            fn,
                            bias=bias_sb[:, hm * 4 + g : hm * 4 + g + 1],
                        ).then_inc(a_sem, 1)
                for n in range(NN):
                    scalar.wait_ge(d_sem, 12 * hm + (3 if n == 0 else 6))
                    nc.scalar.activation(
                        t_tc[:, n, :], t_c[:, n, :], AFT.Tanh
                    ).then_inc(a_sem, 1)

        @block.vector
        def _(vector):
            vector.wait_ge(init_sem, 16 * DMA_INIT)  # cprev tiles loaded
            for hm in range(HMT):
                base = 10 * hm
                for n in range(NN):
                    vector.wait_ge(a_sem, base + (2 if n == 0 else 6))
                    nc.vector.tensor_mul(
                        t_fc[:, n, :], f_sb[:, n, :], cp[:, hm * NN + n, :]
                    ).then_inc(d_sem, 1)
                    vector.wait_ge(a_sem, base + (3 if n == 0 else 7))
                    nc.vector.tensor_mul(
                        t_ig[:, n, :], i_sb[:, n, :], g_sb[:, n, :]
                    ).then_inc(d_sem, 1)
                    nc.vector.tensor_add(
                        t_c[:, n, :], t_fc[:, n, :], t_ig[:, n, :]
                    ).then_inc(d_sem, 1)
                for n in range(NN):
                    vector.wait_ge(a_sem, base + (9 if n == 0 else 10))
                    nc.vector.tensor_mul(
                        t_h[:, n, :], o_sb[:, n, :], t_tc[:, n, :]
                    ).then_inc(d_sem, 1)
                    nc.vector.tensor_add(
                        t_s[:, n, :], t_c[:, n, :], t_h[:, n, :]
                    ).then_inc(d_sem, 1)
                    idx = hm * NN + n
                    nc.vector.reduce_sum(
                        acc_sb[:, idx : idx + 1],
                        t_s[:, n, :],
                        axis=mybir.AxisListType.X,
                    ).then_inc(d_sem, 1)

    nc.compile()
    return nc



def _get_compiled(scheme):
    if scheme not in _compiled:
        if scheme == "bf16x3":
            _compiled[scheme] = _build_raw()
        elif scheme == "bf16x3_tile":
            _compiled[scheme] = _build("bf16x3")
        else:
            _compiled[scheme] = _build(scheme)
    return _compiled[scheme]


def _split_lohi(a, np_dt):
    hi = a.astype(np_dt)
    lo = (a - hi.astype(np.float32)).astype(np_dt)
    return hi, lo


def _prep_inputs(scheme, x, h_prev, c_prev, w_ih, w_hh, b_ih, b_hh):
    import ml_dtypes

    f32 = np.float32
    x = np.asarray(x, f32)
    h_prev = np.asarray(h_prev, f32)
    c_prev = np.asarray(c_prev, f32)
    w_ih = np.asarray(w_ih, f32)
    w_hh = np.asarray(w_hh, f32)
    b = (np.asarray(b_ih, f32) + np.asarray(b_hh, f32)).reshape(4, NCORES, HMT, P)

    xh = np.concatenate([x, h_prev], axis=0)  # [8192, B]

    if scheme == "bf16x3_tile":
        scheme = "bf16x3"
    if scheme in ("fp32", "fp32r"):
        np_dt = f32
    elif scheme in ("bf16x1", "bf16x3"):
        np_dt = ml_dtypes.bfloat16
    elif scheme == "fp16x1":
        np_dt = np.float16
    else:
        raise ValueError(scheme)

    split = scheme.endswith("x3")
    if split:
        xh_hi, xh_lo = _split_lohi(xh, np_dt)
        x_maps = {"xh0": xh_hi, "xh1": xh_lo}
    else:
        x_maps = {"xh0": xh.astype(np_dt)}

    wih_r = w_ih.reshape(4, NCORES, RPC, D)
    whh_r = w_hh.reshape(4, NCORES, RPC, H)

    in_maps = []
    for d in range(NCORES):
        wc = np.concatenate([wih_r[:, d], whh_r[:, d]], axis=2)  # (4, 512, 8192)
        wc = wc.reshape(4, HMT, P, D + H)  # (g, hm, r, k)
        wt = np.ascontiguousarray(wc.transpose(1, 3, 0, 2)).reshape(
            HMT, D + H, 4 * P
        )  # (hm, k, g*128+r)
        m = dict(x_maps)
        if split:
            w_hi, w_lo = _split_lohi(wt, np_dt)
            m["wt0"] = w_hi
            m["wt1"] = w_lo
        else:
            m["wt0"] = wt.astype(np_dt)
        m["cprev"] = np.ascontiguousarray(c_prev[d * RPC : (d + 1) * RPC])
        m["bias"] = np.ascontiguousarray(
            b[:, d].transpose(2, 1, 0).reshape(P, HMT * 4)
        )
        in_maps.append(m)
    return in_maps


def _ensure_axon_ntff_hook():
    """Register the axon NTFF-profile hook if the container's `antenv` stub
    lacks `axon_hooks` (needed only for trace=True / BASS_TRACE runs)."""
    import contextlib
    import ctypes
    import sys
    import types

    try:
        from antenv import axon_hooks  # noqa: F401

        return
    except ImportError:
        pass
    try:
        import antenv
    except ImportError:
        return

    holder = {}
    mod = types.ModuleType("antenv.axon_hooks")
    mod.set_axon_ntff_profile_hook = lambda h: holder.__setitem__("h", h)
    mod.get_axon_ntff_profile_hook = lambda: holder.get("h")
    sys.modules["antenv.axon_hooks"] = mod
    antenv.axon_hooks = mod

    so_path = "/opt/axon/libaxon_pjrt.so"
    try:
        lib = ctypes.CDLL(so_path)
        if not hasattr(lib, "axon_start_nrt_profile"):
            return
        lib.axon_start_nrt_profile.argtypes = [
            ctypes.POINTER(ctypes.c_int64),
            ctypes.c_size_t,
        ]
        lib.axon_start_nrt_profile.restype = ctypes.c_int64
        lib.axon_stop_nrt_profile.argtypes = [ctypes.c_char_p]
        lib.axon_stop_nrt_profile.restype = ctypes.c_int64

        @contextlib.contextmanager
        def _hook(output_dir, device_ids):
            import jax

            jax.devices()
            if device_ids:
                ids = (ctypes.c_int64 * len(device_ids))(*device_ids)
                rc = lib.axon_start_nrt_profile(ids, len(device_ids))
            else:
                rc = lib.axon_start_nrt_profile(None, 0)
            if rc != 0:
                raise RuntimeError(f"axon_start_nrt_profile rc={rc}")
            try:
                yield
            finally:
                n = lib.axon_stop_nrt_profile(str(output_dir).encode())
                print(f"ntff profile: {n} file(s) -> {output_dir}", file=sys.stderr)

        mod.set_axon_ntff_profile_hook(_hook)
    except Exception:
        pass


def kernel(x, h_prev, c_prev, w_ih, w_hh, b_ih, b_hh):
    global LAST_RESULT
    from concourse.bass_utils import run_bass_kernel_spmd

    if os.environ.get("BASS_TRACE"):
        _ensure_axon_ntff_hook()

    scheme = SCHEME
    nc = _get_compiled(scheme)
    in_maps = _prep_inputs(scheme, x, h_prev, c_prev, w_ih, w_hh, b_ih, b_hh)
    res = run_bass_kernel_spmd(nc, in_maps, core_ids=list(range(NCORES)))
    LAST_RESULT = res
    total = np.float64(0.0)
    for r in res.results:
        total += np.asarray(r["out"], np.float64).sum()
    return np.array(total, dtype=np.float32)
